# revision 1
# baseline (speedup 1.0000x reference)
"""BertSelfAttention TRN2 Bass kernel (8-core data-parallel over batch).

Per core (one batch element):
  hidden [T, H] -> hT via PE transposes -> fused QKV projection split by
  orientation (qT/kT feature-major, v token-major), then per-head attention
  entirely in k-on-partitions layout:
    pass 1: raw scores (mask folded in via an aux contraction row) -> exp ->
            column sums S1 via ones-matmul -> c = -ln(S1)  (a valid per-query
            softmax stabilizer: max <= ln S1 <= max + ln(T))
    pass 2: scores recomputed with c injected via a second aux contraction
            row -> exp(8*(s+c)) which is the softmax numerator up to a
            per-query factor -> context matmul with [v | 1] stationary gives
            unnormalized context and the normalizer Z in one accumulation ->
            PE transpose back to [q, d] -> multiply by 1/Z.
All matmuls run in float32r (fast PE mode, fp32 PSUM accumulation).
"""
import sys

sys.path.insert(0, "/opt/trn_rl_repo")

import contextlib

import numpy as np
import concourse.bacc as bacc
import concourse.mybir as mybir
import concourse.tile as tile
from concourse.bass_utils import run_bass_kernel_spmd

F32 = mybir.dt.float32
F32R = mybir.dt.float32r
EXP = mybir.ActivationFunctionType.Exp
LN = mybir.ActivationFunctionType.Ln

HD = 64  # head dim (fixed)


@contextlib.contextmanager
def _single_act_table():
    """During compile, resolve Exp and Ln only from the one table set that
    holds both, so the kernel loads activation tables once instead of
    thrashing between exp_and_others and natural_log_exp_and_others."""
    orig = bacc.get_activation_tables

    def patched(arch):
        tables = orig(arch)
        if "natural_log_exp_and_others" in tables:
            for name, fns in tables.items():
                if name != "natural_log_exp_and_others":
                    fns.discard(mybir.ActivationFunctionType.Exp)
                    fns.discard(mybir.ActivationFunctionType.Ln)
        return tables

    bacc.get_activation_tables = patched
    try:
        yield
    finally:
        bacc.get_activation_tables = orig


def build_module(T, H, NH):
    """One-core program; run SPMD on 8 cores with per-core batch slices."""
    NT = T // 128      # token tiles
    NHT = H // 128     # hidden-dim tiles
    QC = min(512, T)   # query chunk (moving-dim >= 256 keeps f32r at full rate)
    NQC = T // QC

    nc = bacc.Bacc("TRN2", target_bir_lowering=False, debug=False, num_devices=8)

    hidden = nc.dram_tensor("hidden", [T, H], F32R, kind="ExternalInput").ap()
    w = nc.dram_tensor("w", [H, 3 * H], F32R, kind="ExternalInput").ap()
    mask_row = nc.dram_tensor("mask_row", [1, T], F32R, kind="ExternalInput").ap()
    ones_row = nc.dram_tensor("ones_row", [1, T], F32R, kind="ExternalInput").ap()
    neg_row = nc.dram_tensor("neg_row", [1, T], F32R, kind="ExternalInput").ap()
    ones_col = nc.dram_tensor("ones_col", [128, 1], F32R, kind="ExternalInput").ap()
    ones_blk = nc.dram_tensor("ones_blk", [128, NT * NH], F32R, kind="ExternalInput").ap()
    qk_bias = nc.dram_tensor("qk_bias", [128, NH], F32, kind="ExternalInput").ap()
    v_bias = nc.dram_tensor("v_bias", [128, H], F32, kind="ExternalInput").ap()
    ident_r = nc.dram_tensor("ident_r", [128, 128], F32R, kind="ExternalInput").ap()
    ident_f = nc.dram_tensor("ident_f", [128, 128], F32, kind="ExternalInput").ap()
    out = nc.dram_tensor("out", [T, H], F32, kind="ExternalOutput").ap()

    with tile.TileContext(nc) as tc:
        with tc.tile_pool(name="persist", bufs=1) as persist, tc.tile_pool(
            name="work", bufs=2
        ) as work, tc.tile_pool(name="outp", bufs=4) as outp, tc.tile_pool(
            name="psb", bufs=1, space="PSUM"
        ) as psb, tc.tile_pool(name="psc", bufs=4, space="PSUM") as psc, tc.tile_pool(
            name="pss", bufs=2, space="PSUM"
        ) as pss:
            # ---- constants ----
            idr = persist.tile([128, 128], F32R, tag="idr")
            idf = persist.tile([128, 128], F32, tag="idf")
            nc.sync.dma_start(out=idr, in_=ident_r)
            nc.sync.dma_start(out=idf, in_=ident_f)
            onec = persist.tile([128, 1], F32R, tag="onec")
            nc.sync.dma_start(out=onec, in_=ones_col)
            qkb = persist.tile([128, NH], F32, tag="qkb")
            nc.sync.dma_start(out=qkb, in_=qk_bias)
            vb = persist.tile([128, H], F32, tag="vb")
            nc.sync.dma_start(out=vb, in_=v_bias)

            # ---- phase 0: hT[p, ht, t] = hidden[t, ht*128+p] ----
            hT = persist.tile([128, NHT, T], F32R, tag="hT")
            for t in range(NT):
                hid = work.tile([128, H], F32R, tag="hid")
                nc.sync.dma_start(out=hid, in_=hidden[t * 128 : (t + 1) * 128, :])
                for hb in range(NHT):
                    tp = pss.tile([128, 128], F32R, tag="small")
                    nc.tensor.transpose(tp[:], hid[:, hb * 128 : (hb + 1) * 128], idr[:])
                    nc.vector.tensor_copy(hT[:, hb, t * 128 : (t + 1) * 128], tp[:])

            # ---- phase 1: v_aug[p, kt, h, 0:64] = v proj + bias; [.., 64] = 1 ----
            wv = persist.tile([128, NHT, H], F32R, tag="wv")
            for ht in range(NHT):
                wsl = w[ht * 128 : (ht + 1) * 128, :].rearrange(
                    "p (h three d) -> p h three d", three=3, d=HD
                )
                nc.sync.dma_start(
                    out=wv[:, ht, :].rearrange("p (h d) -> p h d", d=HD),
                    in_=wsl[:, :, 2, :],
                )
            v_aug = persist.tile([128, NT, NH, HD + 1], F32R, tag="v_aug")
            nc.sync.dma_start(
                out=v_aug[:, :, :, HD : HD + 1],
                in_=ones_blk.rearrange("p (a b one) -> p a b one", b=NH, one=1),
            )
            VW = min(512, H)
            NVH = VW // HD
            for t in range(NT):
                for half in range(H // VW):
                    vp = psc.tile([128, VW], F32, tag="sc")
                    for ht in range(NHT):
                        nc.tensor.matmul(
                            vp[:],
                            hT[:, ht, t * 128 : (t + 1) * 128],
                            wv[:, ht, half * VW : (half + 1) * VW],
                            start=(ht == 0),
                            stop=(ht == NHT - 1),
                        )
                    nc.vector.tensor_add(
                        v_aug[:, t, half * NVH : (half + 1) * NVH, 0:HD],
                        vp[:].rearrange("p (h d) -> p h d", d=HD),
                        vb[:, half * VW : (half + 1) * VW].rearrange(
                            "p (h d) -> p h d", d=HD
                        ),
                    )

            # ---- per-head attention ----
            for h in range(NH):
                # fused q|k projection for this head: psum [128f(q0:64,k64:128), T]
                wqk = work.tile([128, NHT, 128], F32R, tag="wqk")
                nc.sync.dma_start(
                    out=wqk,
                    in_=w[:, h * 3 * HD : h * 3 * HD + 128].rearrange(
                        "(ht p) f -> p ht f", p=128
                    ),
                )
                qkp = psb.tile([128, T], F32, tag="pj")
                for ht in range(NHT):
                    for half in range(NQC):
                        nc.tensor.matmul(
                            qkp[:, half * QC : (half + 1) * QC],
                            wqk[:, ht, :],
                            hT[:, ht, half * QC : (half + 1) * QC],
                            start=(ht == 0),
                            stop=(ht == NHT - 1),
                        )
                qaux = work.tile([66, T], F32R, tag="qaux")
                nc.vector.tensor_scalar_add(qaux[0:64, :], qkp[0:64, :], qkb[0:64, h : h + 1])
                ktmp = work.tile([128, T], F32R, tag="ktmp")
                nc.vector.tensor_scalar_add(
                    ktmp[64:128, :], qkp[64:128, :], qkb[64:128, h : h + 1]
                )
                kaux = work.tile([66, T], F32R, tag="kaux")
                nc.sync.dma_start(out=kaux[0:64, :], in_=ktmp[64:128, :])
                nc.sync.dma_start(out=kaux[64:65, :], in_=mask_row)
                # row 65 pairs with qaux's c row (= +ln S1_half); -2 injects
                # -2*ln(S1_half), a valid stabilizer in [max, max+2*ln T]
                nc.sync.dma_start(out=kaux[65:66, :], in_=neg_row)
                nc.sync.dma_start(out=qaux[64:65, :], in_=ones_row)

                # pass 1: S1 sums -> c = -ln(S1) per query
                csb = work.tile([1, T], F32R, tag="csb")
                for qc in range(NQC):
                    qs = qaux[0:65, qc * QC : (qc + 1) * QC]
                    s1p = pss.tile([1, QC], F32, tag="small")
                    for kt in range(NT):
                        sp = psc.tile([128, QC], F32, tag="sc")
                        e1 = work.tile([128, QC], F32R, tag="e1", bufs=3)
                        nc.tensor.matmul(
                            sp[:],
                            kaux[0:65, kt * 128 : (kt + 1) * 128],
                            qs,
                            start=True,
                            stop=True,
                        )
                        # half-scale keeps S1 <= e^29.5, inside ACT Ln's
                        # valid input range (Ln breaks above ~2^64)
                        nc.scalar.activation(out=e1[:], in_=sp[:], func=EXP, scale=0.5)
                        nc.tensor.matmul(
                            s1p[:],
                            onec[:],
                            e1[:],
                            start=(kt == 0),
                            stop=(kt == NT - 1),
                        )
                    nc.scalar.activation(
                        out=csb[:, qc * QC : (qc + 1) * QC],
                        in_=s1p[:],
                        func=LN,
                        scale=1.0,
                    )
                nc.sync.dma_start(out=qaux[65:66, :], in_=csb[:])

                # pass 2: e8 = exp(8*(raw - 1e4*m - lnS1)) ; ctx/Z accumulate
                ctxT = work.tile([65, T], F32, tag="ctxT")
                for qc in range(NQC):
                    qs = qaux[0:66, qc * QC : (qc + 1) * QC]
                    cxp = pss.tile([65, QC], F32, tag="small")
                    for kt in range(NT):
                        sp = psc.tile([128, QC], F32, tag="sc")
                        e8 = work.tile([128, QC], F32R, tag="e8", bufs=3)
                        nc.tensor.matmul(
                            sp[:],
                            kaux[0:66, kt * 128 : (kt + 1) * 128],
                            qs,
                            start=True,
                            stop=True,
                        )
                        nc.scalar.activation(out=e8[:], in_=sp[:], func=EXP, scale=8.0)
                        nc.tensor.matmul(
                            cxp[:],
                            v_aug[:, kt, h, :],
                            e8[:],
                            start=(kt == 0),
                            stop=(kt == NT - 1),
                        )
                    nc.vector.tensor_copy(ctxT[:, qc * QC : (qc + 1) * QC], cxp[:])

                # transpose back per query tile, normalize by Z, write out
                for qt in range(NT):
                    trp = pss.tile([128, 65], F32, tag="small")
                    nc.tensor.transpose(
                        trp[:], ctxT[:, qt * 128 : (qt + 1) * 128], idf[0:65, 0:65]
                    )
                    rz = outp.tile([128, 1], F32, tag="rz")
                    nc.vector.reciprocal(rz[:], trp[:, 64:65])
                    ot = outp.tile([128, HD], F32, tag="ot")
                    nc.vector.tensor_scalar_mul(ot[:], trp[:, 0:64], rz[:])
                    nc.sync.dma_start(
                        out=out[qt * 128 : (qt + 1) * 128, h * HD : (h + 1) * HD],
                        in_=ot,
                    )

    with _single_act_table():
        nc.compile()
    return nc


_module_cache = {}


def _get_module(T, H, NH):
    key = (T, H, NH)
    if key not in _module_cache:
        _module_cache[key] = build_module(T, H, NH)
    return _module_cache[key]


def run_sharded(hidden_states, attention_mask, w_qkv, b_qkv, trace=False):
    B, T, H = hidden_states.shape
    NH = H // HD
    NT = T // 128
    nc = _get_module(T, H, NH)

    w_np = np.ascontiguousarray(w_qkv.astype(np.float32))
    b_np = np.asarray(b_qkv, dtype=np.float32)
    # qk_bias[p, h] = b[h*192 + p]  (q bias rows 0-63, k bias rows 64-127)
    qkb = np.empty((128, NH), np.float32)
    for h in range(NH):
        qkb[:, h] = b_np[h * 3 * HD : h * 3 * HD + 128]
    # v_bias broadcast [128, H]
    vb_row = np.empty((H,), np.float32)
    for h in range(NH):
        vb_row[h * HD : (h + 1) * HD] = b_np[h * 3 * HD + 2 * HD : h * 3 * HD + 3 * HD]
    vb = np.broadcast_to(vb_row, (128, H)).copy()
    ones_row = np.ones((1, T), np.float32)
    neg_row = np.full((1, T), -2.0, np.float32)
    ones_col = np.ones((128, 1), np.float32)
    ones_blk = np.ones((128, NT * NH), np.float32)
    ident = np.eye(128, dtype=np.float32)

    in_maps = []
    for b in range(B):
        m = np.asarray(attention_mask[b]).reshape(-1).astype(np.float32)
        in_maps.append(
            dict(
                hidden=np.ascontiguousarray(hidden_states[b].astype(np.float32)),
                w=w_np,
                mask_row=(m * np.float32(-10000.0)).reshape(1, T),
                ones_row=ones_row,
                neg_row=neg_row,
                ones_col=ones_col,
                ones_blk=ones_blk,
                qk_bias=qkb,
                v_bias=vb,
                ident_r=ident,
                ident_f=ident,
            )
        )
    res = run_bass_kernel_spmd(nc, in_maps, core_ids=list(range(B)), trace=trace)
    return np.stack([res.results[b]["out"] for b in range(B)]), res


def kernel(hidden_states, attention_mask, w_qkv, b_qkv):
    out, _ = run_sharded(
        np.asarray(hidden_states),
        np.asarray(attention_mask),
        np.asarray(w_qkv),
        np.asarray(b_qkv),
    )
    return out.astype(np.float32)



# revision 22
# speedup vs baseline: 1.0050x; 1.0050x over previous
"""BertSelfAttention TRN2 Bass kernel (8-core data-parallel over batch).

Per core (one batch element), engines balanced against the cost model:
  hidden [T, H] -> hT via PE transposes (ACT copies PSUM->SBUF) -> fused QKV
  projection with q/k biases folded in as rank-1 aux matmuls (PE) and
  extracted straight from PSUM to SBUF by casting software-DGE DMAs, then
  per head:
    pass 1: scores in [q-partitions, k-free] orientation (PE, fp32r) ->
            exact per-query row max via DVE free-dim reduces -> gpsimd
            combines chunk maxes -> tiny PE transpose turns the max column
            into the stabilizer row c.
    pass 2: scores in [k-partitions, q-free] orientation with mask and -c
            injected as aux contraction rows -> ACT exp(8*(s-c)) to bf16 ->
            context matmul restructured as out[q, d+1]: stationary e8
            [k, 128q] x moving [v|1] bf16 emits only 65 columns per matmul
            (vs 512), giving unnormalized context AND the softmax
            normalizer Z directly in [q, d] layout - no output transpose.
  DVE reciprocal + ACT scale-copy normalize; software-DGE DMAs stream out.
"""
import sys

sys.path.insert(0, "/opt/trn_rl_repo")

import numpy as np
import ml_dtypes
import concourse.bacc as bacc
import concourse.mybir as mybir
import concourse.tile as tile
from concourse.bass_utils import run_bass_kernel_spmd

F32 = mybir.dt.float32
F32R = mybir.dt.float32r
BF16 = mybir.dt.bfloat16
EXP = mybir.ActivationFunctionType.Exp
AX = mybir.AxisListType.X

HD = 64  # head dim (fixed)


def build_module(T, H, NH, dbg=0):
    """One-core program; run SPMD on 8 cores with per-core batch slices."""
    NT = T // 128      # token tiles
    NHT = H // 128     # hidden-dim tiles
    QC = min(512, T)   # chunk size (moving-dim >= 256 keeps f32r at full rate)
    NQC = T // QC
    JT = QC // 128     # q-subtiles per chunk

    nc = bacc.Bacc("TRN2", target_bir_lowering=False, debug=False, num_devices=8)

    if dbg:
        c_rows = nc.dram_tensor("c_rows", [NH, T], F32R, kind="ExternalInput").ap()
        dbg_q = nc.dram_tensor("dbg_q", [NH * 66, T], F32, kind="ExternalOutput").ap()
        dbg_k = nc.dram_tensor("dbg_k", [NH * 66, T], F32, kind="ExternalOutput").ap()
        dbg_v = nc.dram_tensor(
            "dbg_v", [128, NT * NH * (HD + 1)], BF16, kind="ExternalOutput"
        ).ap()

    hidden = nc.dram_tensor("hidden", [T, H], F32R, kind="ExternalInput").ap()
    w = nc.dram_tensor("w", [H, 3 * H], F32R, kind="ExternalInput").ap()
    mask_row = nc.dram_tensor("mask_row", [1, T], F32R, kind="ExternalInput").ap()
    ones_row = nc.dram_tensor("ones_row", [1, T], F32R, kind="ExternalInput").ap()
    neg_row = nc.dram_tensor("neg_row", [1, T], F32R, kind="ExternalInput").ap()
    qk_bias = nc.dram_tensor("qk_bias", [128, NH], F32, kind="ExternalInput").ap()
    v_bias = nc.dram_tensor("v_bias", [128, H], F32, kind="ExternalInput").ap()
    ident_r = nc.dram_tensor("ident_r", [128, 128], F32R, kind="ExternalInput").ap()
    ident_f = nc.dram_tensor("ident_f", [128, 128], F32, kind="ExternalInput").ap()
    out = nc.dram_tensor("out", [T, H], F32, kind="ExternalOutput").ap()

    with tile.TileContext(nc) as tc:
        with tc.tile_pool(name="persist", bufs=1) as persist:
            # ---- constants ----
            idr = persist.tile([128, 128], F32R, tag="idr")
            nc.sync.dma_start(out=idr, in_=ident_r)
            idf = persist.tile([128, 128], F32, tag="idf")
            nc.sync.dma_start(out=idf, in_=ident_f)
            qkb_sb = persist.tile([128, NH], F32, tag="qkb_sb")
            nc.sync.dma_start(out=qkb_sb, in_=qk_bias)
            onesr_sb = persist.tile([1, T], F32R, tag="onesr_sb")
            nc.sync.dma_start(out=onesr_sb, in_=ones_row)
            mask_sb = persist.tile([1, T], F32R, tag="mask_sb")
            nc.sync.dma_start(out=mask_sb, in_=mask_row)
            negr_sb = persist.tile([1, T], F32R, tag="negr_sb")
            nc.sync.dma_start(out=negr_sb, in_=neg_row)
            vb = persist.tile([128, H], F32, tag="vb")
            nc.sync.dma_start(out=vb, in_=v_bias)

            hT = persist.tile([128, NHT, T], F32R, tag="hT")
            wv = persist.tile([128, NHT, H], F32R, tag="wv")
            v_aug = persist.tile([128, NT, NH, HD + 1], BF16, tag="v_aug")

            # ---- phase 0 + 1 in their own psum scope (released after) ----
            with tc.tile_pool(name="ppro", bufs=2, space="PSUM") as ppro, tc.tile_pool(
                name="wpro", bufs=2
            ) as wpro:
                # hT[p, ht, t] = hidden[t, ht*128+p]
                for t in range(NT):
                    hid = wpro.tile([128, H], F32R, tag="hid")
                    nc.sync.dma_start(out=hid, in_=hidden[t * 128 : (t + 1) * 128, :])
                    for hb in range(NHT):
                        tp = ppro.tile([128, 128], F32R, tag="tp", bufs=3)
                        nc.tensor.transpose(tp[:], hid[:, hb * 128 : (hb + 1) * 128], idr[:])
                        nc.scalar.copy(hT[:, hb, t * 128 : (t + 1) * 128], tp[:])

                # v_aug[p, kt, h, 0:64] = v proj + bias; [.., 64] = 1
                for ht in range(NHT):
                    wsl = w[ht * 128 : (ht + 1) * 128, :].rearrange(
                        "p (h three d) -> p h three d", three=3, d=HD
                    )
                    nc.sync.dma_start(
                        out=wv[:, ht, :].rearrange("p (h d) -> p h d", d=HD),
                        in_=wsl[:, :, 2, :],
                    )
                nc.vector.memset(v_aug[:, :, :, HD : HD + 1], 1.0)
                VW = min(512, H)
                NVH = VW // HD
                for half in range(H // VW):
                    for t in range(NT):
                        vp = ppro.tile([128, VW], F32, tag="vp")
                        for ht in range(NHT):
                            nc.tensor.matmul(
                                vp[:],
                                hT[:, ht, t * 128 : (t + 1) * 128],
                                wv[:, ht, half * VW : (half + 1) * VW],
                                start=(ht == 0),
                                stop=(ht == NHT - 1),
                            )
                        nc.vector.tensor_add(
                            v_aug[:, t, half * NVH : (half + 1) * NVH, 0:HD],
                            vp[:].rearrange("p (h d) -> p h d", d=HD),
                            vb[:, half * VW : (half + 1) * VW].rearrange(
                                "p (h d) -> p h d", d=HD
                            ),
                        )

            if dbg:
                nc.sync.dma_start(
                    out=dbg_v, in_=v_aug.rearrange("p a b c -> p (a b c)")
                )
            # ---- per-head attention ----
            with tc.tile_pool(name="work", bufs=2) as work, tc.tile_pool(
                name="small", bufs=4
            ) as small, tc.tile_pool(name="e8p", bufs=NT + 2) as e8p, tc.tile_pool(
                name="outp", bufs=4
            ) as outp, tc.tile_pool(
                name="psb", bufs=2, space="PSUM"
            ) as psb, tc.tile_pool(name="ps1", bufs=2, space="PSUM") as ps1, tc.tile_pool(
                name="ps2", bufs=2, space="PSUM"
            ) as ps2, tc.tile_pool(name="pcx", bufs=2, space="PSUM") as pcx:
                for h in range(NH):
                    # fused q|k projection, biases folded in as rank-1 matmuls;
                    # biased q/k extracted from PSUM by casting SWDGE DMAs
                    wqk = work.tile([128, NHT, 128], F32R, tag="wqk")
                    nc.sync.dma_start(
                        out=wqk,
                        in_=w[:, h * 3 * HD : h * 3 * HD + 128].rearrange(
                            "(ht p) f -> p ht f", p=128
                        ),
                    )
                    qaux = work.tile([66, T], F32R, tag="qaux")
                    kaux = work.tile([66, T], F32R, tag="kaux")
                    ktmp = work.tile([128, T], F32R, tag="ktmp")
                    for half in range(NQC):
                        sl = slice(half * QC, (half + 1) * QC)
                        qkp = psb.tile([128, QC], F32, tag="qkp")
                        for ht in range(NHT):
                            nc.tensor.matmul(
                                qkp[:],
                                wqk[:, ht, :],
                                hT[:, ht, sl],
                                start=(ht == 0),
                                stop=(ht == NHT - 1),
                            )
                        # q gets its bias folded in here; the k bias only adds
                        # a per-query constant to every logit (softmax
                        # invariant), so k is extracted as a plain copy
                        nc.vector.tensor_scalar_add(
                            qaux[0:64, sl], qkp[0:64, :], qkb_sb[0:64, h : h + 1]
                        )
                        nc.scalar.copy(ktmp[64:128, sl], qkp[64:128, :])
                    nc.gpsimd.dma_start(out=kaux[0:64, :], in_=ktmp[64:128, :])
                    nc.gpsimd.dma_start(out=qaux[64:65, :], in_=onesr_sb)
                    nc.gpsimd.dma_start(out=kaux[64:65, :], in_=mask_sb)
                    nc.gpsimd.dma_start(out=kaux[65:66, :], in_=negr_sb)

                    # pass 1: exact row max in [q, k] orientation (DVE reduces)
                    mcol = work.tile([128, NT], F32, tag="mcol")
                    for qt in range(NT):
                        qsl = qaux[0:65, qt * 128 : (qt + 1) * 128]
                        if NQC == 1:
                            s1 = ps1.tile([128, QC], F32, tag="s1")
                            nc.tensor.matmul(
                                s1[:], qsl, kaux[0:65, :], start=True, stop=True
                            )
                            nc.vector.reduce_max(mcol[:, qt : qt + 1], s1[:], axis=AX)
                        else:
                            m0 = small.tile([128, 1], F32, tag="m0")
                            m1 = small.tile([128, 1], F32, tag="m1")
                            for kc in range(NQC):
                                s1 = ps1.tile([128, QC], F32, tag="s1")
                                nc.tensor.matmul(
                                    s1[:],
                                    qsl,
                                    kaux[0:65, kc * QC : (kc + 1) * QC],
                                    start=True,
                                    stop=True,
                                )
                                nc.vector.reduce_max(
                                    (m0 if kc == 0 else m1)[:], s1[:], axis=AX
                                )
                            nc.vector.tensor_max(mcol[:, qt : qt + 1], m0[:], m1[:])
                    # stabilizer column -> row: tiny PE transpose, ACT evicts
                    # to SBUF, DMA flattens partition-major into the c row
                    ctr = pcx.tile([128, 512], F32, tag="cx")
                    nc.tensor.transpose(ctr[0:NT, 0:128], mcol[:], idf[:])
                    crow_sb = small.tile([NT, 128], F32R, tag="crow_sb")
                    nc.scalar.copy(crow_sb[:], ctr[0:NT, 0:128])
                    # per-q-tile row DMAs: single source partition each, so the
                    # element order is unambiguous on real DMA engines
                    for qt in range(NT):
                        nc.gpsimd.dma_start(
                            out=qaux[65:66, qt * 128 : (qt + 1) * 128],
                            in_=crow_sb[qt : qt + 1, :],
                        )
                    if dbg:
                        if dbg == 2:  # override stabilizer with host-computed c
                            nc.gpsimd.dma_start(
                                out=qaux[65:66, :], in_=c_rows[h : h + 1, :]
                            )
                        nc.gpsimd.dma_start(out=dbg_q[h * 66 : (h + 1) * 66, :], in_=qaux)
                        nc.gpsimd.dma_start(out=dbg_k[h * 66 : (h + 1) * 66, :], in_=kaux)

                    # pass 2: e8 = exp(8*(s - 1e4*m - c)); ctx/Z in [q, d] layout
                    for qc in range(NQC):
                        e8s = []
                        for kt in range(NT):
                            sp = ps2.tile([128, QC], F32, tag="sp")
                            nc.tensor.matmul(
                                sp[:],
                                kaux[0:66, kt * 128 : (kt + 1) * 128],
                                qaux[0:66, qc * QC : (qc + 1) * QC],
                                start=True,
                                stop=True,
                            )
                            e8 = e8p.tile([128, QC], BF16, tag="e8")
                            nc.scalar.activation(out=e8[:], in_=sp[:], func=EXP, scale=8.0)
                            e8s.append(e8)
                        for j in range(JT):
                            qt = qc * JT + j
                            cx = pcx.tile([128, 512], F32, tag="cx")
                            for kt in range(NT):
                                nc.tensor.matmul(
                                    cx[:, 0 : HD + 1],
                                    e8s[kt][:, j * 128 : (j + 1) * 128],
                                    v_aug[:, kt, h, :],
                                    start=(kt == 0),
                                    stop=(kt == NT - 1),
                                )
                            rz = outp.tile([128, 1], F32, tag="rz")
                            nc.vector.reciprocal(rz[:], cx[:, HD : HD + 1])
                            ot = outp.tile([128, HD], F32, tag="ot")
                            nc.scalar.mul(ot[:], cx[:, 0:HD], rz[:])
                            nc.gpsimd.dma_start(
                                out=out[qt * 128 : (qt + 1) * 128, h * HD : (h + 1) * HD],
                                in_=ot,
                            )

    nc.compile()
    return nc


_module_cache = {}


def _get_module(T, H, NH, dbg=0):
    key = (T, H, NH, dbg)
    if key not in _module_cache:
        _module_cache[key] = build_module(T, H, NH, dbg)
    return _module_cache[key]


def run_sharded(hidden_states, attention_mask, w_qkv, b_qkv, trace=False):
    B, T, H = hidden_states.shape
    NH = H // HD
    NT = T // 128
    nc = _get_module(T, H, NH)

    w_np = np.ascontiguousarray(w_qkv.astype(np.float32))
    b_np = np.asarray(b_qkv, dtype=np.float32)
    # qk_bias[p, h] = b[h*192 + p] (q bias rows 0-63; k bias unused - it only
    # shifts every logit of a query by a constant, which softmax cancels)
    qkb = np.empty((128, NH), np.float32)
    for h in range(NH):
        qkb[:, h] = b_np[h * 3 * HD : h * 3 * HD + 128]
    # v_bias broadcast [128, H]
    vb_row = np.empty((H,), np.float32)
    for h in range(NH):
        vb_row[h * HD : (h + 1) * HD] = b_np[h * 3 * HD + 2 * HD : h * 3 * HD + 3 * HD]
    vb = np.broadcast_to(vb_row, (128, H)).copy()
    ones_row = np.ones((1, T), np.float32)
    neg_row = np.full((1, T), -1.0, np.float32)
    ident = np.eye(128, dtype=np.float32)

    in_maps = []
    for b in range(B):
        m = np.asarray(attention_mask[b]).reshape(-1).astype(np.float32)
        in_maps.append(
            dict(
                hidden=np.ascontiguousarray(hidden_states[b].astype(np.float32)),
                w=w_np,
                mask_row=(m * np.float32(-10000.0)).reshape(1, T),
                ones_row=ones_row,
                neg_row=neg_row,
                qk_bias=qkb,
                v_bias=vb,
                ident_r=ident,
                ident_f=ident,
            )
        )
    res = run_bass_kernel_spmd(nc, in_maps, core_ids=list(range(B)), trace=trace)
    return np.stack([res.results[b]["out"] for b in range(B)]), res


def kernel(hidden_states, attention_mask, w_qkv, b_qkv):
    out, _ = run_sharded(
        np.asarray(hidden_states),
        np.asarray(attention_mask),
        np.asarray(w_qkv),
        np.asarray(b_qkv),
    )
    return out.astype(np.float32)


# revision 42
# speedup vs baseline: 1.3389x; 1.3323x over previous
"""BertSelfAttention TRN2 Bass kernel (8-core data-parallel over batch).

Per core (one batch element), engines balanced against the cost model:
  hidden [T, H] -> hT via PE transposes -> fused QKV projection into a
  [128, T] PSUM ring; q extracted with its bias by one DVE op, k by one ACT
  copy + partition-shift DMA (the k bias only adds a per-query constant to
  every logit, which softmax cancels, so it is dropped).  Per head:
    pass 1: scores in [q-partitions, k-free] orientation (PE, fp32r) into a
            [128, T] PSUM tile -> one DVE reduce gives the exact per-query
            row max -> tiny PE transpose + DRAM round-trip turns the max
            column into the stabilizer row c (element order via DRAM
            linearization is unambiguous).
    pass 2: scores in [k-partitions, q-free] orientation with mask and -c
            injected as aux contraction rows -> ACT exp(8*(s-c)) to bf16 ->
            context matmul restructured as out[q, d+1]: stationary e8
            [k, 128q] x moving [v|1] bf16 emits only 65 columns per matmul,
            giving unnormalized context AND the softmax normalizer Z
            directly in [q, d] layout - no output transpose.
  The emission software-pipelines heads: head h+1's projection and pass 1
  are zipped between head h's pass-2 score matmuls so PE always has ready
  work while ACT paces the exps and DVE paces the reduces.
"""
import sys

sys.path.insert(0, "/opt/trn_rl_repo")

from collections import deque

import numpy as np
import concourse.bacc as bacc
import concourse.mybir as mybir
import concourse.tile as tile
from concourse.bass_utils import run_bass_kernel_spmd

F32 = mybir.dt.float32
F32R = mybir.dt.float32r
BF16 = mybir.dt.bfloat16
EXP = mybir.ActivationFunctionType.Exp
AX = mybir.AxisListType.X

HD = 64  # head dim (fixed)


def build_module(T, H, NH, dbg=0):
    """One-core program; run SPMD on 8 cores with per-core batch slices."""
    NT = T // 128      # token tiles
    NHT = H // 128     # hidden-dim tiles
    QC = min(512, T)   # chunk size (moving-dim >= 256 keeps f32r at full rate)
    NQC = T // QC
    JT = QC // 128     # q-subtiles per chunk

    nc = bacc.Bacc("TRN2", target_bir_lowering=False, debug=False, num_devices=8)

    if dbg:
        c_rows = nc.dram_tensor("c_rows", [NH, T], F32R, kind="ExternalInput").ap()
        dbg_q = nc.dram_tensor("dbg_q", [NH * 66, T], F32, kind="ExternalOutput").ap()
        dbg_k = nc.dram_tensor("dbg_k", [NH * 66, T], F32, kind="ExternalOutput").ap()

    hidden = nc.dram_tensor("hidden", [T, H], F32, kind="ExternalInput").ap()
    w = nc.dram_tensor("w", [H, 3 * H], F32R, kind="ExternalInput").ap()
    mask_row = nc.dram_tensor("mask_row", [1, T], F32R, kind="ExternalInput").ap()
    ones_row = nc.dram_tensor("ones_row", [1, T], F32R, kind="ExternalInput").ap()
    neg_row = nc.dram_tensor("neg_row", [1, T], F32R, kind="ExternalInput").ap()
    qk_bias = nc.dram_tensor("qk_bias", [128, NH], F32, kind="ExternalInput").ap()
    v_bias = nc.dram_tensor("v_bias", [128, H], F32, kind="ExternalInput").ap()
    ident_f = nc.dram_tensor("ident_f", [128, 128], F32, kind="ExternalInput").ap()
    out = nc.dram_tensor("out", [T, H], F32, kind="ExternalOutput").ap()

    with tile.TileContext(nc) as tc:
        with tc.tile_pool(name="persist", bufs=1) as persist, tc.tile_pool(
            name="work", bufs=2
        ) as work, tc.tile_pool(name="small", bufs=4) as small, tc.tile_pool(
            name="e8p", bufs=2 * NT + 2
        ) as e8p, tc.tile_pool(name="outp", bufs=4) as outp, tc.tile_pool(
            name="pbig", bufs=2, space="PSUM"
        ) as pbig, tc.tile_pool(
            name="psp", bufs=3, space="PSUM"
        ) as psp, tc.tile_pool(name="pcx", bufs=1, space="PSUM") as pcx:
            # ---- constants ----
            idf = persist.tile([128, 128], F32, tag="idf")
            nc.sync.dma_start(out=idf, in_=ident_f)

            hT = persist.tile([128, NHT, T], F32R, tag="hT")
            wv = persist.tile([128, NHT, H], F32R, tag="wv")
            v_aug = persist.tile([128, NT, NH, HD + 1], BF16, tag="v_aug")
            nc.vector.memset(v_aug[:, :, :, HD : HD + 1], 1.0)

            # ---- phase 0: hT[p, ht, t] = hidden[t, ht*128+p] ----
            # (hidden loads go out first so the transposes start ASAP)
            hids = []
            for t in range(NT):
                hid = work.tile([128, H], F32, tag="hid", bufs=3, name="hid")
                nc.sync.dma_start(out=hid, in_=hidden[t * 128 : (t + 1) * 128, :])
                hids.append(hid)

            qkb_sb = persist.tile([128, NH], F32, tag="qkb_sb")
            nc.sync.dma_start(out=qkb_sb, in_=qk_bias)
            vb = persist.tile([128, H], F32, tag="vb")
            nc.sync.dma_start(out=vb, in_=v_bias)

            # persistent triple-buffered q/k staging: aux rows written once;
            # by head h the h-3 reader is long done, so per-head writes never
            # stall a DMA queue on a WAR hazard
            qauxs, kauxs, ktmps = [], [], []
            for i in range(3):
                qx = persist.tile([66, T], F32R, tag=f"qaux{i}", name=f"qaux{i}")
                kx = persist.tile([66, T], F32R, tag=f"kaux{i}", name=f"kaux{i}")
                kt_ = persist.tile([128, T], F32R, tag=f"ktmp{i}", name=f"ktmp{i}")
                nc.sync.dma_start(out=qx[64:65, :], in_=ones_row)
                nc.sync.dma_start(out=kx[64:65, :], in_=mask_row)
                nc.sync.dma_start(out=kx[65:66, :], in_=neg_row)
                qauxs.append(qx)
                kauxs.append(kx)
                ktmps.append(kt_)

            for t in range(NT):
                hid = hids[t]
                for hb in range(NHT):
                    tp = psp.tile([128, QC], F32, tag="sp", name="tp")
                    nc.tensor.transpose(
                        tp[:, 0:128], hid[:, hb * 128 : (hb + 1) * 128], idf[:]
                    )
                    nc.vector.tensor_copy(
                        hT[:, hb, t * 128 : (t + 1) * 128], tp[:, 0:128]
                    )

            mcols = {}

            qkps = {}

            def emit_wqk_dma(h):
                wqk = work.tile([128, NHT, 128], F32R, tag="wqk", bufs=3, name="wqk")
                nc.sync.dma_start(
                    out=wqk,
                    in_=w[:, h * 3 * HD : h * 3 * HD + 128].rearrange(
                        "(ht p) f -> p ht f", p=128
                    ),
                )
                return wqk

            def emit_proj_mm(h, wqk, half):
                # fused q|k projection matmuls only; extraction is emitted
                # later so the in-order ACT/DVE streams never stall on it
                sl = slice(half * QC, (half + 1) * QC)
                if half == 0:
                    qkps[h] = pbig.tile([128, T], F32, tag="big", name="qkp")
                qkp = qkps[h]
                for ht in range(NHT):
                    nc.tensor.matmul(
                        qkp[:, sl],
                        wqk[:, ht, :],
                        hT[:, ht, sl],
                        start=(ht == 0),
                        stop=(ht == NHT - 1),
                    )

            def emit_extract(h):
                # q gets its bias here (one DVE op); k is a plain ACT copy +
                # partition-shift DMA
                qaux, kaux, ktmp = qauxs[h % 3], kauxs[h % 3], ktmps[h % 3]
                qkp = qkps.pop(h)
                nc.vector.tensor_scalar_add(
                    qaux[0:64, :], qkp[0:64, :], qkb_sb[0:64, h : h + 1]
                )
                nc.scalar.copy(ktmp[64:128, :], qkp[64:128, :])
                nc.gpsimd.dma_start(out=kaux[0:64, :], in_=ktmp[64:128, :])

            def emit_p1(h, qts):
                # pass 1: exact row max in [q, k] orientation; the two score
                # chunks get max-combined and row-reduced by one fused DVE op
                qaux, kaux = qauxs[h % 3], kauxs[h % 3]
                if h not in mcols:
                    mcols[h] = work.tile([128, NT], F32, tag="mcol", name="mcol")
                mcol = mcols[h]
                for qt in qts:
                    qsl = qaux[0:65, qt * 128 : (qt + 1) * 128]
                    s1 = pbig.tile([128, T], F32, tag="big", name="s1")
                    for kc in range(NQC):
                        nc.tensor.matmul(
                            s1[:, kc * QC : (kc + 1) * QC],
                            qsl,
                            kaux[0:65, kc * QC : (kc + 1) * QC],
                            start=True,
                            stop=True,
                        )
                    nc.vector.reduce_max(mcol[:, qt : qt + 1], s1[:], axis=AX)

            ctrs = {}

            def emit_crow_t(h, half):
                # stabilizer columns -> row pieces: tiny PE transpose first
                mcol = mcols[h]
                HT2 = NT - NT // 2 if half else NT // 2
                q0 = 0 if not half else NT // 2
                ctr = psp.tile([128, QC], F32, tag="sp", name="ctr")
                nc.tensor.transpose(
                    ctr[0:HT2, 0:128], mcol[:, q0 : q0 + HT2], idf[:]
                )
                ctrs[(h, half)] = ctr

            def emit_crow(h, half):
                # ... then the ACT evict + one single-partition DMA per q-tile
                # (element order is trivially unambiguous), emitted a filler
                # later so ACT never parks on the transpose
                qaux, kaux = qauxs[h % 3], kauxs[h % 3]
                HT2 = NT - NT // 2 if half else NT // 2
                q0 = 0 if not half else NT // 2
                ctr = ctrs.pop((h, half))
                crow_sb = small.tile([NT, 128], F32R, tag="crow_sb", name="crow_sb")
                nc.scalar.copy(crow_sb[0:HT2, :], ctr[0:HT2, 0:128])
                for i in range(HT2):
                    nc.sync.dma_start(
                        out=qaux[65:66, (q0 + i) * 128 : (q0 + i + 1) * 128],
                        in_=crow_sb[i : i + 1, :],
                    )
                if dbg and half == (0 if NT == 1 else 1):
                    if dbg == 2:  # override stabilizer with host-computed c
                        nc.sync.dma_start(out=qaux[65:66, :], in_=c_rows[h : h + 1, :])
                    nc.sync.dma_start(out=dbg_q[h * 66 : (h + 1) * 66, :], in_=qaux)
                    nc.sync.dma_start(out=dbg_k[h * 66 : (h + 1) * 66, :], in_=kaux)

            def emit_sc(h, qc, kt):
                # one pass-2 score matmul + exp tile
                qaux, kaux = qauxs[h % 3], kauxs[h % 3]
                sp = psp.tile([128, QC], F32, tag="sp", name="sp")
                nc.tensor.matmul(
                    sp[:],
                    kaux[0:66, kt * 128 : (kt + 1) * 128],
                    qaux[0:66, qc * QC : (qc + 1) * QC],
                    start=True,
                    stop=True,
                )
                e8 = e8p.tile([128, QC], BF16, tag="e8", name="e8")
                nc.scalar.activation(out=e8[:], in_=sp[:], func=EXP, scale=8.0)
                return e8

            def emit_ctx(h, qc, e8s):
                # context matmuls emit [q, d|Z] directly; normalize trails
                cxs = []
                for j in range(JT):
                    if len(cxs) == 2:
                        emit_norm(h, qc, j - 2, cxs.pop(0))
                    cx = pcx.tile([128, 512], F32, tag="cx", name="cx")
                    for kt in range(NT):
                        nc.tensor.matmul(
                            cx[:, 0 : HD + 1],
                            e8s[kt][:, j * 128 : (j + 1) * 128],
                            v_aug[:, kt, h, :],
                            start=(kt == 0),
                            stop=(kt == NT - 1),
                        )
                    cxs.append(cx)
                for i, cx in enumerate(cxs):
                    emit_norm(h, qc, JT - len(cxs) + i, cx)

            def emit_norm(h, qc, j, cx):
                qt = qc * JT + j
                rz = outp.tile([128, 1], F32, tag="rz", name="rz")
                nc.vector.reciprocal(rz[:], cx[:, HD : HD + 1])
                ot = outp.tile([128, HD], F32, tag="ot", name="ot")
                nc.vector.tensor_scalar_mul(ot[:], cx[:, 0:HD], rz[:])
                eng = nc.sync if (qt % 2 == 0) else nc.gpsimd
                eng.dma_start(
                    out=out[qt * 128 : (qt + 1) * 128, h * HD : (h + 1) * HD],
                    in_=ot,
                )

            # head 0 staged up front, head 1's projection primed; the loop
            # then runs a 2-deep stage pipeline: during head h's pass-2 the
            # fillers run pass-1 of head h+1 AND projection/extract of h+2,
            # so the stage latency chain spans two periods
            wqk0 = emit_wqk_dma(0)
            for half in range(NQC):
                emit_proj_mm(0, wqk0, half)
            emit_extract(0)
            emit_p1(0, range(NT))
            emit_crow_t(0, 0)
            emit_crow(0, 0)
            if NT > 1:
                emit_crow_t(0, 1)
                emit_crow(0, 1)
            wqks = {}
            for g in range(1, min(3, NH)):
                wqks[g] = emit_wqk_dma(g)
            if NH > 1:
                for half in range(NQC):
                    emit_proj_mm(1, wqks[1], half)
                emit_extract(1)

            VW = min(512, H)
            NVH = VW // HD
            for ht in range(NHT):
                wsl = w[ht * 128 : (ht + 1) * 128, :].rearrange(
                    "p (h three d) -> p h three d", three=3, d=HD
                )
                nc.sync.dma_start(
                    out=wv[:, ht, :].rearrange("p (h d) -> p h d", d=HD),
                    in_=wsl[:, :, 2, :],
                )

            def emit_vproj(half, ts_):
                for t in ts_:
                    vp = psp.tile([128, QC], F32, tag="sp", name="vp")
                    for ht in range(NHT):
                        nc.tensor.matmul(
                            vp[:, 0:VW],
                            hT[:, ht, t * 128 : (t + 1) * 128],
                            wv[:, ht, half * VW : (half + 1) * VW],
                            start=(ht == 0),
                            stop=(ht == NHT - 1),
                        )
                    nc.vector.tensor_add(
                        v_aug[:, t, half * NVH : (half + 1) * NVH, 0:HD],
                        vp[:, 0:VW].rearrange("p (h d) -> p h d", d=HD),
                        vb[:, half * VW : (half + 1) * VW].rearrange(
                            "p (h d) -> p h d", d=HD
                        ),
                    )

            # head 0's ctx needs half-0 v columns first: emit that eagerly
            emit_vproj(0, range(NT))

            # ---- software-pipelined head loop (2-deep stages) ----
            pending_ctx = [None]
            for h in range(NH):
                fillers = deque()
                if h + 3 < NH:
                    wqks[h + 3] = emit_wqk_dma(h + 3)
                if h + 2 < NH:
                    wqk = wqks.pop(h + 2)
                    for half in range(NQC):
                        fillers.append(
                            lambda g=h + 2, wqk=wqk, half=half: emit_proj_mm(
                                g, wqk, half
                            )
                        )
                if h + 1 < NH:
                    nh = h + 1
                    HT2 = NT // 2
                    for q0 in range(0, NT, 2):
                        def fill(nh=nh, q0=q0, HT2=HT2):
                            emit_p1(nh, range(q0, min(q0 + 2, NT)))
                            if q0 < HT2 <= q0 + 2 or (HT2 == 0 and q0 == 0):
                                emit_crow_t(nh, 0)
                            elif HT2 < q0 + 2 <= HT2 + 2:
                                emit_crow(nh, 0)
                            if q0 + 2 >= NT and NT > 1:
                                emit_crow_t(nh, 1)
                        fillers.append(fill)
                    def fill_tail(nh=nh, HT2=HT2):
                        if HT2 == 0:
                            return
                        if (nh, 0) in ctrs:
                            emit_crow(nh, 0)
                        if NT > 1:
                            emit_crow(nh, 1)
                    fillers.append(fill_tail)
                if h == 0:
                    for half in range(1, H // VW):
                        for t0 in range(0, NT, 2):
                            fillers.append(
                                lambda half=half, t0=t0: emit_vproj(
                                    half, range(t0, min(t0 + 2, NT))
                                )
                            )
                if h + 2 < NH:
                    # extraction last: by then its projection is long done, so
                    # the in-order ACT/DVE streams never park on it
                    fillers.append(lambda g=h + 2: emit_extract(g))
                for qc in range(NQC):
                    e8s = []
                    for kt in range(NT):
                        e8s.append(emit_sc(h, qc, kt))
                        if kt == 1 and pending_ctx[0] is not None:
                            pending_ctx[0]()
                            pending_ctx[0] = None
                        elif fillers:
                            fillers.popleft()()
                    pc = lambda h=h, qc=qc, e8s=e8s: emit_ctx(h, qc, e8s)
                    if pending_ctx[0] is not None:
                        pending_ctx[0]()
                    pending_ctx[0] = pc
                    if fillers:
                        fillers.popleft()()
                while fillers:
                    fillers.popleft()()
                if h + 1 < NH:
                    mcols.pop(h + 1)
            if pending_ctx[0] is not None:
                pending_ctx[0]()

    nc.compile()
    return nc


_module_cache = {}


def _get_module(T, H, NH, dbg=0):
    key = (T, H, NH, dbg)
    if key not in _module_cache:
        _module_cache[key] = build_module(T, H, NH, dbg)
    return _module_cache[key]


def run_sharded(hidden_states, attention_mask, w_qkv, b_qkv, trace=False):
    B, T, H = hidden_states.shape
    NH = H // HD
    nc = _get_module(T, H, NH)

    w_np = np.ascontiguousarray(w_qkv.astype(np.float32))
    b_np = np.asarray(b_qkv, dtype=np.float32)
    # qk_bias[p, h] = b[h*192 + p] (q bias rows 0-63; k bias unused - it only
    # shifts every logit of a query by a constant, which softmax cancels)
    qkb = np.empty((128, NH), np.float32)
    for h in range(NH):
        qkb[:, h] = b_np[h * 3 * HD : h * 3 * HD + 128]
    # v_bias broadcast [128, H]
    vb_row = np.empty((H,), np.float32)
    for h in range(NH):
        vb_row[h * HD : (h + 1) * HD] = b_np[h * 3 * HD + 2 * HD : h * 3 * HD + 3 * HD]
    vb = np.broadcast_to(vb_row, (128, H)).copy()
    ones_row = np.ones((1, T), np.float32)
    neg_row = np.full((1, T), -1.0, np.float32)
    ident = np.eye(128, dtype=np.float32)

    in_maps = []
    for b in range(B):
        m = np.asarray(attention_mask[b]).reshape(-1).astype(np.float32)
        in_maps.append(
            dict(
                hidden=np.ascontiguousarray(hidden_states[b].astype(np.float32)),
                w=w_np,
                mask_row=(m * np.float32(-10000.0)).reshape(1, T),
                ones_row=ones_row,
                neg_row=neg_row,
                qk_bias=qkb,
                v_bias=vb,
                ident_f=ident,
            )
        )
    res = run_bass_kernel_spmd(nc, in_maps, core_ids=list(range(B)), trace=trace)
    return np.stack([res.results[b]["out"] for b in range(B)]), res


def kernel(hidden_states, attention_mask, w_qkv, b_qkv):
    out, _ = run_sharded(
        np.asarray(hidden_states),
        np.asarray(attention_mask),
        np.asarray(w_qkv),
        np.asarray(b_qkv),
    )
    return out.astype(np.float32)


# revision 43
# speedup vs baseline: 1.3512x; 1.0092x over previous
"""BertSelfAttention TRN2 Bass kernel (8-core data-parallel over batch).

Per core (one batch element), engines balanced against the cost model:
  hidden [T, H] -> hT via PE transposes -> fused QKV projection into a
  [128, T] PSUM ring; q extracted with its bias by one DVE op, k by one ACT
  copy + partition-shift DMA (the k bias only adds a per-query constant to
  every logit, which softmax cancels, so it is dropped).  Per head:
    pass 1: scores in [q-partitions, k-free] orientation (PE, fp32r) into a
            [128, T] PSUM tile -> one DVE reduce gives the exact per-query
            row max -> tiny PE transpose + DRAM round-trip turns the max
            column into the stabilizer row c (element order via DRAM
            linearization is unambiguous).
    pass 2: scores in [k-partitions, q-free] orientation with mask and -c
            injected as aux contraction rows -> ACT exp(8*(s-c)) to bf16 ->
            context matmul restructured as out[q, d+1]: stationary e8
            [k, 128q] x moving [v|1] bf16 emits only 65 columns per matmul,
            giving unnormalized context AND the softmax normalizer Z
            directly in [q, d] layout - no output transpose.
  The emission software-pipelines heads: head h+1's projection and pass 1
  are zipped between head h's pass-2 score matmuls so PE always has ready
  work while ACT paces the exps and DVE paces the reduces.
"""
import sys

sys.path.insert(0, "/opt/trn_rl_repo")

from collections import deque

import numpy as np
import concourse.bacc as bacc
import concourse.mybir as mybir
import concourse.tile as tile
from concourse.bass_utils import run_bass_kernel_spmd

F32 = mybir.dt.float32
F32R = mybir.dt.float32r
BF16 = mybir.dt.bfloat16
EXP = mybir.ActivationFunctionType.Exp
AX = mybir.AxisListType.X

HD = 64  # head dim (fixed)


def build_module(T, H, NH, dbg=0):
    """One-core program; run SPMD on 8 cores with per-core batch slices."""
    NT = T // 128      # token tiles
    NHT = H // 128     # hidden-dim tiles
    QC = min(512, T)   # chunk size (moving-dim >= 256 keeps f32r at full rate)
    NQC = T // QC
    JT = QC // 128     # q-subtiles per chunk

    nc = bacc.Bacc("TRN2", target_bir_lowering=False, debug=False, num_devices=8)

    if dbg:
        c_rows = nc.dram_tensor("c_rows", [NH, T], F32R, kind="ExternalInput").ap()
        dbg_q = nc.dram_tensor("dbg_q", [NH * 66, T], F32, kind="ExternalOutput").ap()
        dbg_k = nc.dram_tensor("dbg_k", [NH * 66, T], F32, kind="ExternalOutput").ap()

    hidden = nc.dram_tensor("hidden", [T, H], F32, kind="ExternalInput").ap()
    w = nc.dram_tensor("w", [H, 3 * H], F32R, kind="ExternalInput").ap()
    mask_row = nc.dram_tensor("mask_row", [1, T], F32R, kind="ExternalInput").ap()
    ones_row = nc.dram_tensor("ones_row", [1, T], F32R, kind="ExternalInput").ap()
    neg_row = nc.dram_tensor("neg_row", [1, T], F32R, kind="ExternalInput").ap()
    qk_bias = nc.dram_tensor("qk_bias", [128, NH], F32, kind="ExternalInput").ap()
    v_bias = nc.dram_tensor("v_bias", [128, H], F32, kind="ExternalInput").ap()
    ident_f = nc.dram_tensor("ident_f", [128, 128], F32, kind="ExternalInput").ap()
    out = nc.dram_tensor("out", [T, H], F32, kind="ExternalOutput").ap()

    with tile.TileContext(nc) as tc:
        with tc.tile_pool(name="persist", bufs=1) as persist, tc.tile_pool(
            name="work", bufs=2
        ) as work, tc.tile_pool(name="small", bufs=4) as small, tc.tile_pool(
            name="e8p", bufs=2 * NT + 2
        ) as e8p, tc.tile_pool(name="outp", bufs=4) as outp, tc.tile_pool(
            name="pbig", bufs=2, space="PSUM"
        ) as pbig, tc.tile_pool(name="psp", bufs=4, space="PSUM") as psp:
            # ---- constants ----
            idf = persist.tile([128, 128], F32, tag="idf")
            nc.sync.dma_start(out=idf, in_=ident_f)

            hT = persist.tile([128, NHT, T], F32R, tag="hT")
            wv = persist.tile([128, NHT, H], F32R, tag="wv")
            v_aug = persist.tile([128, NT, NH, HD + 1], BF16, tag="v_aug")
            nc.vector.memset(v_aug[:, :, :, HD : HD + 1], 1.0)

            # ---- phase 0: hT[p, ht, t] = hidden[t, ht*128+p] ----
            # (hidden loads go out first so the transposes start ASAP)
            hids = []
            for t in range(NT):
                hid = work.tile([128, H], F32, tag="hid", bufs=3, name="hid")
                nc.sync.dma_start(out=hid, in_=hidden[t * 128 : (t + 1) * 128, :])
                hids.append(hid)

            qkb_sb = persist.tile([128, NH], F32, tag="qkb_sb")
            nc.sync.dma_start(out=qkb_sb, in_=qk_bias)
            vb = persist.tile([128, H], F32, tag="vb")
            nc.sync.dma_start(out=vb, in_=v_bias)

            # persistent triple-buffered q/k staging: aux rows written once;
            # by head h the h-3 reader is long done, so per-head writes never
            # stall a DMA queue on a WAR hazard
            qauxs, kauxs, ktmps = [], [], []
            for i in range(3):
                qx = persist.tile([66, T], F32R, tag=f"qaux{i}", name=f"qaux{i}")
                kx = persist.tile([66, T], F32R, tag=f"kaux{i}", name=f"kaux{i}")
                kt_ = persist.tile([128, T], F32R, tag=f"ktmp{i}", name=f"ktmp{i}")
                nc.sync.dma_start(out=qx[64:65, :], in_=ones_row)
                nc.sync.dma_start(out=kx[64:65, :], in_=mask_row)
                nc.sync.dma_start(out=kx[65:66, :], in_=neg_row)
                qauxs.append(qx)
                kauxs.append(kx)
                ktmps.append(kt_)

            for t in range(NT):
                hid = hids[t]
                for hb in range(NHT):
                    tp = psp.tile([128, QC], F32, tag="sp", name="tp")
                    nc.tensor.transpose(
                        tp[:, 0:128], hid[:, hb * 128 : (hb + 1) * 128], idf[:]
                    )
                    if hb % 2 == 0:
                        nc.vector.tensor_copy(
                            hT[:, hb, t * 128 : (t + 1) * 128], tp[:, 0:128]
                        )
                    else:
                        nc.scalar.copy(
                            hT[:, hb, t * 128 : (t + 1) * 128], tp[:, 0:128]
                        )

            mcols = {}

            qkps = {}

            def emit_wqk_dma(h):
                wqk = work.tile([128, NHT, 128], F32R, tag="wqk", bufs=3, name="wqk")
                nc.sync.dma_start(
                    out=wqk,
                    in_=w[:, h * 3 * HD : h * 3 * HD + 128].rearrange(
                        "(ht p) f -> p ht f", p=128
                    ),
                )
                return wqk

            def emit_proj_mm(h, wqk, half):
                # fused q|k projection matmuls only; extraction is emitted
                # later so the in-order ACT/DVE streams never stall on it
                sl = slice(half * QC, (half + 1) * QC)
                if half == 0:
                    qkps[h] = pbig.tile([128, T], F32, tag="big", name="qkp")
                qkp = qkps[h]
                for ht in range(NHT):
                    nc.tensor.matmul(
                        qkp[:, sl],
                        wqk[:, ht, :],
                        hT[:, ht, sl],
                        start=(ht == 0),
                        stop=(ht == NHT - 1),
                    )

            def emit_extract(h):
                # q gets its bias here (one DVE op); k is a plain ACT copy +
                # partition-shift DMA
                qaux, kaux, ktmp = qauxs[h % 3], kauxs[h % 3], ktmps[h % 3]
                qkp = qkps.pop(h)
                nc.vector.tensor_scalar_add(
                    qaux[0:64, :], qkp[0:64, :], qkb_sb[0:64, h : h + 1]
                )
                nc.scalar.copy(ktmp[64:128, :], qkp[64:128, :])
                nc.gpsimd.dma_start(out=kaux[0:64, :], in_=ktmp[64:128, :])

            def emit_p1(h, qts):
                # pass 1: exact row max in [q, k] orientation; the two score
                # chunks get max-combined and row-reduced by one fused DVE op
                qaux, kaux = qauxs[h % 3], kauxs[h % 3]
                if h not in mcols:
                    mcols[h] = work.tile([128, NT], F32, tag="mcol", name="mcol")
                mcol = mcols[h]
                for qt in qts:
                    qsl = qaux[0:65, qt * 128 : (qt + 1) * 128]
                    s1 = pbig.tile([128, T], F32, tag="big", name="s1")
                    for kc in range(NQC):
                        nc.tensor.matmul(
                            s1[:, kc * QC : (kc + 1) * QC],
                            qsl,
                            kaux[0:65, kc * QC : (kc + 1) * QC],
                            start=True,
                            stop=True,
                        )
                    nc.vector.reduce_max(mcol[:, qt : qt + 1], s1[:], axis=AX)

            ctrs = {}

            def emit_crow_t(h, half):
                # stabilizer columns -> row pieces: tiny PE transpose first
                mcol = mcols[h]
                HT2 = NT - NT // 2 if half else NT // 2
                q0 = 0 if not half else NT // 2
                ctr = psp.tile([128, QC], F32, tag="sp", name="ctr")
                nc.tensor.transpose(
                    ctr[0:HT2, 0:128], mcol[:, q0 : q0 + HT2], idf[:]
                )
                ctrs[(h, half)] = ctr

            def emit_crow(h, half):
                # ... then the ACT evict + one single-partition DMA per q-tile
                # (element order is trivially unambiguous), emitted a filler
                # later so ACT never parks on the transpose
                qaux, kaux = qauxs[h % 3], kauxs[h % 3]
                HT2 = NT - NT // 2 if half else NT // 2
                q0 = 0 if not half else NT // 2
                ctr = ctrs.pop((h, half))
                crow_sb = small.tile([NT, 128], F32R, tag="crow_sb", name="crow_sb")
                nc.scalar.copy(crow_sb[0:HT2, :], ctr[0:HT2, 0:128])
                for i in range(HT2):
                    nc.sync.dma_start(
                        out=qaux[65:66, (q0 + i) * 128 : (q0 + i + 1) * 128],
                        in_=crow_sb[i : i + 1, :],
                    )
                if dbg and half == (0 if NT == 1 else 1):
                    if dbg == 2:  # override stabilizer with host-computed c
                        nc.sync.dma_start(out=qaux[65:66, :], in_=c_rows[h : h + 1, :])
                    nc.sync.dma_start(out=dbg_q[h * 66 : (h + 1) * 66, :], in_=qaux)
                    nc.sync.dma_start(out=dbg_k[h * 66 : (h + 1) * 66, :], in_=kaux)

            def emit_sc(h, qc, kt):
                # one pass-2 score matmul + exp tile
                qaux, kaux = qauxs[h % 3], kauxs[h % 3]
                sp = psp.tile([128, QC], F32, tag="sp", name="sp")
                nc.tensor.matmul(
                    sp[:],
                    kaux[0:66, kt * 128 : (kt + 1) * 128],
                    qaux[0:66, qc * QC : (qc + 1) * QC],
                    start=True,
                    stop=True,
                )
                e8 = e8p.tile([128, QC], BF16, tag="e8", name="e8")
                nc.scalar.activation(out=e8[:], in_=sp[:], func=EXP, scale=8.0)
                return e8

            def emit_ctx(h, qc, e8s):
                # context matmuls emit [q, d|Z] directly; normalize trails
                cxs = []
                for j in range(JT):
                    if len(cxs) == 2:
                        emit_norm(h, qc, j - 2, cxs.pop(0))
                    cx = psp.tile([128, QC], F32, tag="sp", name="cx")
                    for kt in range(NT):
                        nc.tensor.matmul(
                            cx[:, 0 : HD + 1],
                            e8s[kt][:, j * 128 : (j + 1) * 128],
                            v_aug[:, kt, h, :],
                            start=(kt == 0),
                            stop=(kt == NT - 1),
                        )
                    cxs.append(cx)
                for i, cx in enumerate(cxs):
                    emit_norm(h, qc, JT - len(cxs) + i, cx)

            def emit_norm(h, qc, j, cx):
                qt = qc * JT + j
                rz = outp.tile([128, 1], F32, tag="rz", name="rz")
                nc.vector.reciprocal(rz[:], cx[:, HD : HD + 1])
                ot = outp.tile([128, HD], F32, tag="ot", name="ot")
                nc.vector.tensor_scalar_mul(ot[:], cx[:, 0:HD], rz[:])
                eng = nc.sync if (qt % 2 == 0) else nc.gpsimd
                eng.dma_start(
                    out=out[qt * 128 : (qt + 1) * 128, h * HD : (h + 1) * HD],
                    in_=ot,
                )

            # head 0 staged up front, head 1's projection primed; the loop
            # then runs a 2-deep stage pipeline: during head h's pass-2 the
            # fillers run pass-1 of head h+1 AND projection/extract of h+2,
            # so the stage latency chain spans two periods
            wqk0 = emit_wqk_dma(0)
            for half in range(NQC):
                emit_proj_mm(0, wqk0, half)
            emit_extract(0)
            emit_p1(0, range(NT))
            emit_crow_t(0, 0)
            emit_crow(0, 0)
            if NT > 1:
                emit_crow_t(0, 1)
                emit_crow(0, 1)
            wqks = {}
            for g in range(1, min(3, NH)):
                wqks[g] = emit_wqk_dma(g)
            if NH > 1:
                for half in range(NQC):
                    emit_proj_mm(1, wqks[1], half)
                emit_extract(1)

            VW = min(512, H)
            NVH = VW // HD
            for ht in range(NHT):
                wsl = w[ht * 128 : (ht + 1) * 128, :].rearrange(
                    "p (h three d) -> p h three d", three=3, d=HD
                )
                nc.sync.dma_start(
                    out=wv[:, ht, :].rearrange("p (h d) -> p h d", d=HD),
                    in_=wsl[:, :, 2, :],
                )

            def emit_vproj(half, ts_):
                for t in ts_:
                    vp = psp.tile([128, QC], F32, tag="sp", name="vp")
                    for ht in range(NHT):
                        nc.tensor.matmul(
                            vp[:, 0:VW],
                            hT[:, ht, t * 128 : (t + 1) * 128],
                            wv[:, ht, half * VW : (half + 1) * VW],
                            start=(ht == 0),
                            stop=(ht == NHT - 1),
                        )
                    nc.vector.tensor_add(
                        v_aug[:, t, half * NVH : (half + 1) * NVH, 0:HD],
                        vp[:, 0:VW].rearrange("p (h d) -> p h d", d=HD),
                        vb[:, half * VW : (half + 1) * VW].rearrange(
                            "p (h d) -> p h d", d=HD
                        ),
                    )

            # head 0's ctx needs half-0 v columns first: emit that eagerly
            emit_vproj(0, range(NT))

            # ---- software-pipelined head loop (2-deep stages) ----
            pending_ctx = [None]
            for h in range(NH):
                fillers = deque()
                if h + 3 < NH:
                    wqks[h + 3] = emit_wqk_dma(h + 3)
                if h + 2 < NH:
                    wqk = wqks.pop(h + 2)
                    for half in range(NQC):
                        fillers.append(
                            lambda g=h + 2, wqk=wqk, half=half: emit_proj_mm(
                                g, wqk, half
                            )
                        )
                if h + 1 < NH:
                    nh = h + 1
                    HT2 = NT // 2
                    for q0 in range(0, NT, 2):
                        def fill(nh=nh, q0=q0, HT2=HT2):
                            emit_p1(nh, range(q0, min(q0 + 2, NT)))
                            if q0 < HT2 <= q0 + 2 or (HT2 == 0 and q0 == 0):
                                emit_crow_t(nh, 0)
                            elif HT2 < q0 + 2 <= HT2 + 2:
                                emit_crow(nh, 0)
                            if q0 + 2 >= NT and NT > 1:
                                emit_crow_t(nh, 1)
                        fillers.append(fill)
                    def fill_tail(nh=nh, HT2=HT2):
                        if HT2 == 0:
                            return
                        if (nh, 0) in ctrs:
                            emit_crow(nh, 0)
                        if NT > 1:
                            emit_crow(nh, 1)
                    fillers.append(fill_tail)
                if h == 0:
                    for half in range(1, H // VW):
                        for t0 in range(0, NT, 2):
                            fillers.append(
                                lambda half=half, t0=t0: emit_vproj(
                                    half, range(t0, min(t0 + 2, NT))
                                )
                            )
                if h + 2 < NH:
                    # extraction last: by then its projection is long done, so
                    # the in-order ACT/DVE streams never park on it
                    fillers.append(lambda g=h + 2: emit_extract(g))
                for qc in range(NQC):
                    e8s = []
                    for kt in range(NT):
                        e8s.append(emit_sc(h, qc, kt))
                        if kt == 1 and pending_ctx[0] is not None:
                            pending_ctx[0]()
                            pending_ctx[0] = None
                        elif fillers:
                            fillers.popleft()()
                    pc = lambda h=h, qc=qc, e8s=e8s: emit_ctx(h, qc, e8s)
                    if pending_ctx[0] is not None:
                        pending_ctx[0]()
                    pending_ctx[0] = pc
                    if fillers:
                        fillers.popleft()()
                while fillers:
                    fillers.popleft()()
                if h + 1 < NH:
                    mcols.pop(h + 1)
            if pending_ctx[0] is not None:
                pending_ctx[0]()

    nc.compile()
    return nc


_module_cache = {}


def _get_module(T, H, NH, dbg=0):
    key = (T, H, NH, dbg)
    if key not in _module_cache:
        _module_cache[key] = build_module(T, H, NH, dbg)
    return _module_cache[key]


def run_sharded(hidden_states, attention_mask, w_qkv, b_qkv, trace=False):
    B, T, H = hidden_states.shape
    NH = H // HD
    nc = _get_module(T, H, NH)

    w_np = np.ascontiguousarray(w_qkv.astype(np.float32))
    b_np = np.asarray(b_qkv, dtype=np.float32)
    # qk_bias[p, h] = b[h*192 + p] (q bias rows 0-63; k bias unused - it only
    # shifts every logit of a query by a constant, which softmax cancels)
    qkb = np.empty((128, NH), np.float32)
    for h in range(NH):
        qkb[:, h] = b_np[h * 3 * HD : h * 3 * HD + 128]
    # v_bias broadcast [128, H]
    vb_row = np.empty((H,), np.float32)
    for h in range(NH):
        vb_row[h * HD : (h + 1) * HD] = b_np[h * 3 * HD + 2 * HD : h * 3 * HD + 3 * HD]
    vb = np.broadcast_to(vb_row, (128, H)).copy()
    ones_row = np.ones((1, T), np.float32)
    neg_row = np.full((1, T), -1.0, np.float32)
    ident = np.eye(128, dtype=np.float32)

    in_maps = []
    for b in range(B):
        m = np.asarray(attention_mask[b]).reshape(-1).astype(np.float32)
        in_maps.append(
            dict(
                hidden=np.ascontiguousarray(hidden_states[b].astype(np.float32)),
                w=w_np,
                mask_row=(m * np.float32(-10000.0)).reshape(1, T),
                ones_row=ones_row,
                neg_row=neg_row,
                qk_bias=qkb,
                v_bias=vb,
                ident_f=ident,
            )
        )
    res = run_bass_kernel_spmd(nc, in_maps, core_ids=list(range(B)), trace=trace)
    return np.stack([res.results[b]["out"] for b in range(B)]), res


def kernel(hidden_states, attention_mask, w_qkv, b_qkv):
    out, _ = run_sharded(
        np.asarray(hidden_states),
        np.asarray(attention_mask),
        np.asarray(w_qkv),
        np.asarray(b_qkv),
    )
    return out.astype(np.float32)
